# revision 6
# baseline (speedup 1.0000x reference)
"""Trainium2 Bass kernel for nn_LossFunction_29145648071076.

Math notes (validated in float64 against the reference; see baseline
docstring for the uplink/noise collapses which are reused here):

  * Q = x x^H is rank-1 (x = sum of comm + sensing beams), so
      gHQg[b,l] = |DUMatInit[b,l]^H x_b|^2   and   P[b,g] = |a_g^H x_b|^2.

  * sum_rate_uu == K = 16 to ~1e-7 bits (Woodbury; rank-1 update), and
    nDU = 1e-9 is < 1 ulp of the ~21 denominator: both handled as in the
    baseline (constant / dropped).

  * a_g is symmetric about 90 deg (sin(g) = sin(180-g)), so P[b,g] =
    P[b,180-g] to ~1e-4 relative: the beampattern reduces over the folded
    91-point grid:
      sum_g P^2          = sum_{g<=90} 2 P^2 - P[90]^2   (host-corrected)
      bfold[g<90]        = b[g] + b[180-g],  bfold[90] = b[90]
      b.P = sum bfold*P,  b.b = sum bfold    (exact)

  * Complex products use a stacked 128-partition contraction:
    X2 col pairs hold [xr;xi] and [xi;-xr], the a_g table holds
    [ar|ai ; ai|-ar], so one f32r matmul yields [Re|Im] of a^H x, and the
    downlink dg = colsum(gq) + nu accumulates the ones-matmul and the
    |CI|^2 matmuls into one PSUM group (den = dg - gq).

  * Everything ships in 2 HBM loads (one [128,377] + one [128,182] f32):
    per-DMA fixed cost on this part is ~2.2us (dispatch 650 + DGE 650 +
    sem-prop 900), so DMA count dominates the old 6-load layout.  The
    host precomputes x (marshalling; the O(B*G*NT) math stays on device)
    and 5 per-core scalar columns ship back for the final combine.

  * Data parallel over batch: B=128 split 16/core across 8 cores.
"""

import numpy as np

B, NT, NR, K, L, M = 128, 64, 64, 16, 16, 8
NCORES = 8
S = B // NCORES          # samples per core
G = 181                  # full beampattern grid
GF = 91                  # folded grid (0..90)
LN2 = float(np.log(2.0))

# main tensor column map
C_TA = 0                 # -ta bias column
C_BLK = 1                # blk bf16-packed (8 f32 cols = 16 bf16 cols)
C_X2 = 9                 # X2 (32 cols): 2s=[xr;xi], 2s+1=[xi;-xr]
C_CI = 41                # CI re/im quad layout (64 cols)
C_PM = 105               # UU power, block-diagonal by sample octet (16)
C_DM = 121               # DUMat [gr;gi] per sample (256 cols)
W_MAIN = 377
W_AG = 2 * GF            # 182

NWARM = 2
_CACHE = {}


def _steering_consts():
    """Folded a_g table, f32 rounding order as the reference."""
    grid = np.linspace(0.0, 180.0, G).astype(np.float32)[:GF]
    n = np.arange(NT, dtype=np.float32)
    sin_t = np.sin(grid * np.float32(np.pi / 180.0)).astype(np.float32)
    phase = (np.float32(np.pi) * sin_t)[:, None] * n          # (GF, NT)
    ar = np.cos(phase).astype(np.float32).T                   # (NT, GF)
    ai = np.sin(phase).astype(np.float32).T
    ag = np.zeros((128, W_AG), np.float32)
    ag[0:64, 0:GF] = ar
    ag[0:64, GF:2 * GF] = ai
    ag[64:128, 0:GF] = ai
    ag[64:128, GF:2 * GF] = -ar
    return np.ascontiguousarray(ag)


def _emit_body(nc, tc, sb, ps, d, mybir):
    AF = mybir.ActivationFunctionType
    OP = mybir.AluOpType
    AX = mybir.AxisListType
    f32 = mybir.dt.float32
    f32r = mybir.dt.float32r
    bf16 = mybir.dt.bfloat16

    # ---- t~0: ACT table preload (Ln set also serves Abs/Square/Copy),
    # DVE memsets, both input DMAs, Pool iota, PE clock warmup ----
    t_dl = sb.tile([1, 1], f32)
    nc.vector.memset(t_dl[:], 0.0)
    nc.scalar.activation(t_dl[:], t_dl[:], AF.Ln, bias=1.0)

    t_wsrc = sb.tile([64, 128], bf16)
    nc.vector.memset(t_wsrc[:], 0.0)
    t_onem = sb.tile([16, 16], f32)
    nc.vector.memset(t_onem[:], 1.0)

    t_main = sb.tile([128, W_MAIN], f32)
    nc.sync.dma_start(t_main[:], d["main"][:])
    t_ag = sb.tile([128, W_AG], f32)
    nc.sync.dma_start(t_ag[:], d["ag"][:])

    t_grid = sb.tile([128, G], f32)
    nc.gpsimd.iota(t_grid[:], [[1, G]], channel_multiplier=0,
                   allow_small_or_imprecise_dtypes=True)

    p_warm = ps.tile([1, 128], f32)
    for _ in range(NWARM):
        nc.tensor.matmul(p_warm[:], t_wsrc[:, 0:1], t_wsrc[:])

    # ---- views into the packed main tile ----
    t_ta = t_main[:, C_TA:C_TA + 1]
    t_blk = t_main[:, C_BLK:C_BLK + 8].bitcast(bf16)          # (128,16)
    X2 = t_main[:, C_X2:C_X2 + 32]
    t_civ = t_main[:, C_CI:C_CI + 64]
    t_pm = t_main[:, C_PM:C_PM + 16]

    # output partials: [sp2c | bp | bb | lnr | P90]
    t_fin = sb.tile([16, 5], f32)

    # ---- gx: per-sample complex <g, x> = [reg | img] (PE) ----
    p_gx = ps.tile([16, 32], f32)
    for s in range(S):
        nc.tensor.matmul(
            p_gx[:, 2 * s:2 * s + 2],
            t_main[:, C_DM + 16 * s:C_DM + 16 * s + 16],
            X2[:, 2 * s:2 * s + 2])

    # ---- mask distance + CI^2 (ACT), ci fold (Pool), indicator (DVE) ----
    t_d = sb.tile([128, G], f32)
    nc.scalar.activation(t_d[:], t_grid[:], AF.Abs, bias=t_ta)
    t_cis = sb.tile([128, 64], f32)
    nc.scalar.activation(t_cis[:], t_civ, AF.Square)
    t_ci2 = sb.tile([128, 32], f32)
    civ4 = t_cis[:].rearrange("p (j c l) -> p j c l", j=2, c=2)
    ci2v = t_ci2[:].rearrange("p (j l) -> p j l", j=2)
    nc.gpsimd.tensor_add(ci2v[:], civ4[:, :, 0], civ4[:, :, 1])
    t_ind = sb.tile([128, G], bf16)
    nc.vector.tensor_scalar(t_ind[:], t_d[:], 10.0, None, op0=OP.is_le)

    # ---- P = |a^H x|^2 path: one f32r matmul -> [Re | Im] (PE) ----
    p_ri = ps.tile([16, 2 * GF], f32)
    nc.tensor.matmul(p_ri[:], X2[:, 0:32:2].bitcast(f32r),
                     t_ag[:].bitcast(f32r))
    # mask count matmul (bf16, exact: counts <= 8)
    p_cnt = ps.tile([16, G], f32)
    nc.tensor.matmul(p_cnt[:], t_blk, t_ind[:])

    # ---- gq = reg^2 + img^2 (ACT square from PSUM, Pool pair-add) ----
    t_gsq = sb.tile([16, 32], f32)
    nc.scalar.activation(t_gsq[:], p_gx[:], AF.Square)
    gsv = t_gsq[:].rearrange("p (s c) -> p s c", c=2)
    t_gq = sb.tile([16, 16], f32)
    nc.gpsimd.tensor_add(t_gq[:], gsv[:, :, 0], gsv[:, :, 1])

    # ---- P^2 (ACT), b indicator (DVE) ----
    t_psq = sb.tile([16, 2 * GF], f32)
    nc.scalar.activation(t_psq[:], p_ri[:], AF.Square)
    t_b = sb.tile([16, G], f32)
    nc.vector.tensor_scalar(t_b[:], p_cnt[:], 0.5, None, op0=OP.is_ge)
    # bb = sum_g b[g] over the full grid (== sum of bfold)
    nc.vector.tensor_reduce(t_fin[:, 2:3], t_b[:], axis=AX.X, op=OP.add)

    # ---- dg = colsum(gq) + nu in one PSUM accumulation group (PE) ----
    p_dg = ps.tile([16, 16], f32)
    nc.tensor.matmul(p_dg[:], t_onem[:], t_gq[:], start=True, stop=False,
                     skip_group_check=True)
    nc.tensor.matmul(p_dg[:, 0:8], t_ci2[:, 0:16], t_pm[:, 0:8],
                     start=False, stop=False, skip_group_check=True)
    nc.tensor.matmul(p_dg[:, 8:16], t_ci2[:, 16:32], t_pm[:, 8:16],
                     start=False, stop=True, skip_group_check=True)

    # ---- downlink tail: den = dg - gq, two Lns, accumulate ----
    t_den = sb.tile([16, 16], f32)
    nc.vector.scalar_tensor_tensor(
        t_den[:], p_dg[:], 1.0, t_gq[:], op0=OP.mult, op1=OP.subtract)
    t_lng = sb.tile([16, 16], f32)
    nc.scalar.activation(t_lng[:], p_dg[:], AF.Ln)
    t_lnd = sb.tile([16, 16], f32)
    nc.scalar.activation(t_lnd[:], t_den[:], AF.Ln)

    # ---- beampattern tail ----
    t_pp = sb.tile([16, GF], f32)
    nc.vector.tensor_add(t_pp[:], t_psq[:, 0:GF], t_psq[:, GF:2 * GF])
    nc.vector.tensor_copy(t_fin[:, 4:5], t_pp[:, 90:91])
    t_scr1 = sb.tile([16, GF], f32)
    nc.vector.scalar_tensor_tensor(
        t_scr1[:], t_pp[:], 2.0, t_pp[:], op0=OP.mult, op1=OP.mult,
        accum_out=t_fin[:, 0:1])
    # bfold: b[g] + b[180-g] for g<90; center col = b[90]
    t_bf = sb.tile([16, GF], f32)
    nc.gpsimd.tensor_add(t_bf[:, 0:90], t_b[:, 0:90], t_b[:, 180:90:-1])
    nc.gpsimd.tensor_copy(t_bf[:, 90:91], t_b[:, 90:91])
    t_scr2 = sb.tile([16, GF], f32)
    nc.vector.scalar_tensor_tensor(
        t_scr2[:], t_bf[:], 1.0, t_pp[:], op0=OP.mult, op1=OP.mult,
        accum_out=t_fin[:, 1:2])
    t_lnr = sb.tile([16, 16], f32)
    nc.vector.scalar_tensor_tensor(
        t_lnr[:], t_lng[:], 1.0, t_lnd[:], op0=OP.mult, op1=OP.subtract,
        accum_out=t_fin[:, 3:4])

    # ---- store per-sample partials; host does the final combine ----
    nc.sync.dma_start(d["out"][:], t_fin[:])


def _declare_drams(nc, mybir, suffix=""):
    f32 = mybir.dt.float32
    return {
        "main": nc.dram_tensor("main" + suffix, [128, W_MAIN], f32,
                               kind="ExternalInput"),
        "ag": nc.dram_tensor("ag" + suffix, [128, W_AG], f32,
                             kind="ExternalInput"),
        "out": nc.dram_tensor("out" + suffix, [16, 5], f32,
                              kind="ExternalOutput"),
    }


def _build_nc():
    import concourse.bass as bass
    import concourse.tile as tile
    from concourse import bacc, mybir

    nc = bacc.Bacc("TRN2", target_bir_lowering=False, debug=False)
    d = _declare_drams(nc, mybir)
    with tile.TileContext(nc) as tc:
        with (
            tc.tile_pool(name="sb", bufs=1) as sb,
            tc.tile_pool(name="ps", bufs=1, space=bass.MemorySpace.PSUM) as ps,
        ):
            _emit_body(nc, tc, sb, ps, d, mybir)
    nc.compile()
    return nc


def _host_prep(inputs):
    DUCom = np.asarray(inputs["DUComMat"])      # (B,L,NT) c64
    Sens = np.asarray(inputs["SensingMat"])     # (B,M,NT) c64
    DUMat = np.asarray(inputs["DUMatInit"])     # (B,L,NT) c64
    TAMat = np.asarray(inputs["TAMatInit"])     # (B,M,2) c64
    CI = np.asarray(inputs["CIMatInit"])        # (B,K,L) c64
    P = np.asarray(inputs["UUPowerMat"])        # (B,K) f32

    agT = _steering_consts()

    x = (DUCom.sum(axis=1) + Sens.sum(axis=1)).astype(np.complex64)  # (B,NT)
    xr = x.real.astype(np.float32)
    xi = x.imag.astype(np.float32)

    # blk (target-to-sample map) as packed bf16
    blk = np.zeros((128, 16), np.float32)
    for s in range(S):
        blk[8 * s:8 * s + 8, s] = 1.0
    u = (blk.view(np.uint32) >> 16).astype(np.uint32).reshape(128, 8, 2)
    blk_packed = (u[:, :, 0] | (u[:, :, 1] << 16)).view(np.float32)

    in_maps = []
    for c in range(NCORES):
        gs = slice(c * S, (c + 1) * S)
        main = np.zeros((128, W_MAIN), np.float32)
        # -ta per target (partition t = 8s + m)
        main[:, C_TA] = -TAMat[gs][:, :, 0].real.astype(np.float32).reshape(-1)
        main[:, C_BLK:C_BLK + 8] = blk_packed
        # X2
        xrc, xic = xr[gs], xi[gs]                              # (S,64)
        main[0:64, C_X2:C_X2 + 32:2] = xrc.T
        main[64:128, C_X2:C_X2 + 32:2] = xic.T
        main[0:64, C_X2 + 1:C_X2 + 32:2] = xic.T
        main[64:128, C_X2 + 1:C_X2 + 32:2] = -xrc.T
        # CI quad + pm
        ci = CI[gs]                                            # (S,16,16)
        for j in range(2):
            blkci = ci[8 * j:8 * j + 8]                        # (8,16,16)
            main[:, C_CI + 32 * j:C_CI + 32 * j + 16] = \
                blkci.real.astype(np.float32).reshape(128, 16)
            main[:, C_CI + 32 * j + 16:C_CI + 32 * j + 32] = \
                blkci.imag.astype(np.float32).reshape(128, 16)
            for cc in range(8):
                main[16 * cc:16 * cc + 16, C_PM + 8 * j + cc] = P[gs][8 * j + cc]
        # DUMat
        dm = DUMat[gs]                                         # (S,16,64)
        main[0:64, C_DM:C_DM + 256] = \
            dm.real.astype(np.float32).transpose(2, 0, 1).reshape(64, 256)
        main[64:128, C_DM:C_DM + 256] = \
            dm.imag.astype(np.float32).transpose(2, 0, 1).reshape(64, 256)

        in_maps.append({
            "main": np.ascontiguousarray(main),
            "ag": agT,
        })
    return in_maps


def kernel(**inputs):
    from concourse.bass_utils import run_bass_kernel_spmd

    if "nc" not in _CACHE:
        _CACHE["nc"] = _build_nc()
    nc = _CACHE["nc"]

    in_maps = _host_prep(inputs)
    res = run_bass_kernel_spmd(nc, in_maps, core_ids=list(range(NCORES)))
    parts = np.array([res.results[c]["out"] for c in range(NCORES)],
                     dtype=np.float64)                         # (8,16,5)
    sp2c = parts[:, :, 0]
    bp = parts[:, :, 1]
    bb = parts[:, :, 2]
    lnr = parts[:, :, 3]
    p90 = parts[:, :, 4]
    lb = sp2c - p90 * p90 - bp * bp / (bb + 1e-10)
    loss = 100.0 * lb.sum() / (G * B) - lnr.sum() / (B * LN2) - 16.0
    return np.float32(loss)
